# revision 10
# baseline (speedup 1.0000x reference)
"""MicroDLRM (hash-embedding DLRM) on 8 TRN2 NeuronCores.

Strategy (per sharding hint): row-shard the 512MB embedding table across the
8 cores (262144 rows each); the host computes the splitmix64 hash (routing
metadata) and buckets each lookup to its owning (core, 32K-row window); each
core gathers its rows from its own HBM table shard with one dma_gather per
window (int16 window-local indices, -1-padded so the Q7 ucode trims the tail).
A second data-parallel pass computes the bot/top MLPs on a 2048-row batch
shard per core in bf16 (f32 accumulate).  The host only does hashing, index
routing, and the inter-pass permutation (the all-to-all of the hint), never
touching table payload beyond shipping contiguous shards.
"""

import sys

sys.path.insert(0, "/opt/trn_rl_repo")

from contextlib import ExitStack

import ml_dtypes
import numpy as np

import concourse.bass as bass
import concourse.bacc as bacc
import concourse.mybir as mybir
from concourse.bass_utils import run_bass_kernel_spmd
from concourse import library_config

N_CORES = 8
V = 2_000_000
D = 64                      # embedding width
B = 16384
N_FEAT = 3
B_SH = B // N_CORES         # 2048 batch rows per core
ROWS_PER_CORE = 262144      # 8 x 262144 = 2,097,152 >= V (last core padded)
WIN = 32768                 # int16-addressable window
N_WIN = ROWS_PER_CORE // WIN
N_GQ = 4             # gather SWDGE queues to round-robin (Q7 pairs)
BF16 = mybir.dt.bfloat16
F32 = mybir.dt.float32

_C1 = np.int64(13787848793156543929 - (1 << 64))
_C2 = np.int64(10723151780598845931 - (1 << 64))


def _hash_mod(idx: np.ndarray, seed: np.int64, mod: int) -> np.ndarray:
    with np.errstate(over="ignore"):
        x = idx.astype(np.int64) ^ seed
        x = (x ^ (x >> np.int64(30))) * _C1
        x = (x ^ (x >> np.int64(27))) * _C2
        x = x ^ (x >> np.int64(31))
    return np.abs(x) % np.int64(mod)


# ---------------------------------------------------------------- pass 1 ----


def _build_gather_nc(nk: tuple):
    """Per core: for window w, gather nk[w]*128 rows of D floats from the
    window's 32K-row slice of the local table shard into SBUF, convert to
    bf16 (halves the store traffic), then store to DRAM.  nk[w] =
    ceil(max-over-cores bucket count / 128); bucket tails are -1-padded so
    the Q7 ucode trims the trailing descriptors (no pad HBM reads).
    idx layout per dma_gather contract: idx j of a window lives at
    [j % 16, j // 16] in 16 partitions, replicated 8x across the 128
    partitions (one copy per Q7 core pair)."""
    nc = bacc.Bacc("TRN2", target_bir_lowering=False, num_devices=N_CORES,
                   debug=False, num_swdge_queues=N_GQ)
    caps = [k * 128 for k in nk]
    ioff = [0]          # idx column offsets per window
    goff = [0]          # gather-out free-dim offsets per window (in elems)
    for k in nk:
        ioff.append(ioff[-1] + k * 8)      # cap/16 columns
        goff.append(goff[-1] + k * D)      # cap/128 slots * D
    table = nc.declare_dram_parameter(
        "table", [ROWS_PER_CORE, D], F32, isOutput=False)
    idx = nc.declare_dram_parameter(
        "idx", [128, ioff[-1]], mybir.dt.int16, isOutput=False)
    gout = nc.declare_dram_parameter(
        "gout", [128, goff[-1]], BF16, isOutput=True)

    with ExitStack() as ctx:
        e = ctx.enter_context
        idx_s = e(nc.sbuf_tensor([128, ioff[-1]], mybir.dt.int16))
        gt = e(nc.sbuf_tensor([128, goff[-1]], F32))
        gtb = e(nc.sbuf_tensor([128, goff[-1]], BF16))
        i_sems = [e(nc.semaphore(f"i_sem{w}")) for w in range(N_WIN)]
        g_sems = [e(nc.semaphore(f"g_sem{w}")) for w in range(N_WIN)]
        v_sems = [e(nc.semaphore(f"v_sem{w}")) for w in range(N_WIN)]
        s_sem = e(nc.semaphore("s_sem"))
        block = e(nc.Block())
        tab_ap = table.ap()

        def gview(ap, w):
            return bass.AP(ap.tensor, ap.offset + goff[w],
                           [ap.ap[0], [D, nk[w]], [1, D]])

        @block.gpsimd
        def _(gpsimd):
            gpsimd.load_library(library_config.mlp)
            for w in range(N_WIN):
                gpsimd.wait_ge(i_sems[w], 16)
                gpsimd.dma_gather(
                    out_ap=gview(gt.ap(), w),
                    in_ap=tab_ap[w * WIN:(w + 1) * WIN, :],
                    idxs_ap=idx_s[:, ioff[w]:ioff[w + 1]],
                    num_idxs=caps[w],
                    num_idxs_reg=caps[w],
                    elem_size=D,
                    queue_num=w % N_GQ,
                ).then_inc(g_sems[w], 16)

        @block.vector
        def _(vector):
            for w in range(0, N_WIN, 2):
                vector.wait_ge(g_sems[w], 16)
                vector.tensor_scalar_add(
                    gtb[:, goff[w]:goff[w + 1]],
                    gt[:, goff[w]:goff[w + 1]], 0.0).then_inc(v_sems[w], 1)

        @block.scalar
        def _(scalar):
            for w in range(1, N_WIN, 2):
                scalar.wait_ge(g_sems[w], 16)
                scalar.copy(gtb[:, goff[w]:goff[w + 1]],
                            gt[:, goff[w]:goff[w + 1]]).then_inc(v_sems[w], 1)

        @block.sync
        def _(sync):
            for w in range(N_WIN):
                sync.dma_start(out=idx_s[:, ioff[w]:ioff[w + 1]],
                               in_=idx.ap()[:, ioff[w]:ioff[w + 1]]
                               ).then_inc(i_sems[w], 16)
            for w in range(N_WIN):
                sync.wait_ge(v_sems[w], 1)
                sync.dma_start(out=gview(gout.ap(), w),
                               in_=gview(gtb.ap(), w)).then_inc(s_sem, 16)
            sync.wait_ge(s_sem, 16 * N_WIN)

    nc.compile()
    return nc


# ---------------------------------------------------------------- pass 2 ----


def _build_mlp_nc(adt=BF16):
    """Per core: bot MLP on dense_x.T shard, concat with embT, top MLP.
    Activations kept as [feature, batch] in bf16; weights stationary on PE;
    f32 PSUM accumulate; bias+relu on alternating Scalar/Vector engines.
    PE stream is software-pipelined across the four 512-column tiles, weights
    arrive as two packed slabs, and a dummy sigmoid preloads the ACT table."""
    nc = bacc.Bacc("TRN2", target_bir_lowering=False, num_devices=N_CORES,
                   debug=False)
    # biases ride in the weight slab (bitcast to f32 cols 98+) so one DMA
    # delivers all weights+biases; xt arrives in parallel on the sync queue
    wcols = 108 if adt == BF16 else 103
    xt = nc.declare_dram_parameter("xt", [16, B_SH], adt, isOutput=False)
    embT = nc.declare_dram_parameter(
        "embT", [N_FEAT * D, B_SH], adt, isOutput=False)
    wslab = nc.declare_dram_parameter(
        "wslab", [128, wcols], adt, isOutput=False)
    out = nc.declare_dram_parameter("out", [1, B_SH], F32, isOutput=True)

    NT = B_SH // 512  # 4 column tiles

    with ExitStack() as ctx:
        e = ctx.enter_context
        xt_s = e(nc.sbuf_tensor([16, B_SH], adt))
        zA = e(nc.sbuf_tensor([128, B_SH], adt))  # 0:8 bot out, 8:128 emb
        zB = e(nc.sbuf_tensor([72, B_SH], adt))   # emb rows 120:192
        h0 = e(nc.sbuf_tensor([8, B_SH], adt))
        t1o = e(nc.sbuf_tensor([32, B_SH], adt))
        t2o = e(nc.sbuf_tensor([16, B_SH], adt))
        o_s = e(nc.sbuf_tensor([1, B_SH], F32))
        ws = e(nc.sbuf_tensor([128, wcols], adt))
        bs = ws  # biases live in the tail columns of the weight slab
        scr = e(nc.sbuf_tensor([1, 2], F32))
        pb0 = e(nc.psum_tensor("pb0", [128, 512], F32))
        pb1 = e(nc.psum_tensor("pb1", [128, 512], F32))
        pl1a = e(nc.psum_tensor("pl1a", [128, 512], F32))
        pl1b = e(nc.psum_tensor("pl1b", [128, 512], F32))
        pl2 = e(nc.psum_tensor("pl2", [128, 512], F32))
        pl3 = e(nc.psum_tensor("pl3", [128, 512], F32))
        d1_sem = e(nc.semaphore("d1_sem"))
        e_sems = [e(nc.semaphore(f"e_sem{t}")) for t in range(NT)]
        z_sem = e(nc.semaphore("z_sem"))
        mm_sem = e(nc.semaphore("mm_sem"))
        as_sem = e(nc.semaphore("as_sem"))   # scalar acts: a1 x4, a3/a5 mix
        vs_sem = e(nc.semaphore("vs_sem"))   # vector acts: a2 x4, a4 x4
        o_sem = e(nc.semaphore("o_sem"))
        block = e(nc.Block())

        wb0 = ws[0:16, 0:8]
        wb1 = ws[0:8, 8:16]
        tw0a = ws[0:128, 16:48]
        tw0b = ws[0:72, 48:80]
        tw1 = ws[0:32, 80:96]
        tw2 = ws[0:16, 96:97]
        if adt == BF16:
            def bias(p, k):  # f32 bias k bitcast from bf16 cols 98+2k
                return bs[0:p, 98 + 2 * k:100 + 2 * k].bitcast(F32)
        else:
            def bias(p, k):
                return bs[0:p, 98 + k:99 + k]
        bb0 = bias(8, 0)
        bb1 = bias(8, 1)
        tb0 = bias(32, 2)
        tb1 = bias(16, 3)
        tb2 = bias(1, 4)

        C = [slice(t * 512, (t + 1) * 512) for t in range(NT)]
        pl1 = [pl1a, pl1b, pl1a, pl1b]
        # PE emission order and 1-based position of each matmul
        pe_order = ([x for t in range(NT) for x in (('b0', t), ('b1', t))]
                    + [('l1a', 0), ('l1b', 0), ('l1a', 1), ('l1b', 1),
                       ('l2', 0), ('l1a', 2), ('l1b', 2), ('l2', 1),
                       ('l3', 0), ('l1a', 3), ('l1b', 3), ('l2', 2),
                       ('l3', 1), ('l2', 3), ('l3', 2), ('l3', 3)])
        mm_pos = {op: i + 1 for i, op in enumerate(pe_order)}
        act_order = ([('a1', t) for t in range(NT)]
                     + [('a3', 0), ('a3', 1), ('a5', 0), ('a3', 2),
                        ('a3', 3), ('a5', 1), ('a5', 2), ('a5', 3)])
        as_pos = {op: i + 1 for i, op in enumerate(act_order)}
        dve_order = ([('a2', t) for t in range(NT)]
                     + [('a4', t) for t in range(NT)])
        vs_pos = {op: i + 1 for i, op in enumerate(dve_order)}

        @block.gpsimd
        def _(gpsimd):
            gpsimd.memset(scr[:], 0.0).then_inc(z_sem, 1)

        @block.sync
        def _(sync):
            sync.dma_start(out=xt_s[:], in_=xt[:]).then_inc(d1_sem, 16)
            for t in range(NT):
                sync.dma_start(out=zA[8:128, C[t]],
                               in_=embT[0:120, C[t]]).then_inc(e_sems[t], 16)
                sync.dma_start(out=zB[:, C[t]],
                               in_=embT[120:192, C[t]]).then_inc(e_sems[t], 16)
            for t in range(NT):
                sync.wait_ge(as_sem, as_pos[('a5', t)])
                sync.dma_start(out=out.ap()[:, C[t]],
                               in_=o_s[:, C[t]]).then_inc(o_sem, 16)
            sync.wait_ge(o_sem, 16 * NT)

        @block.tensor
        def _(tensor):
            tensor.wait_ge(d1_sem, 2 * 16)
            for op in pe_order:
                kind, t = op
                if kind == 'b0':
                    if t > 0:
                        tensor.wait_ge(as_sem, as_pos[('a1', t - 1)])
                    tensor.matmul(pb0.ap()[0:8, :], wb0, xt_s[:, C[t]],
                                  start=True, stop=True).then_inc(mm_sem)
                elif kind == 'b1':
                    tensor.wait_ge(as_sem, as_pos[('a1', t)])
                    if t > 0:
                        tensor.wait_ge(vs_sem, vs_pos[('a2', t - 1)])
                    tensor.matmul(pb1.ap()[0:8, :], wb1, h0[:, C[t]],
                                  start=True, stop=True).then_inc(mm_sem)
                elif kind == 'l1a':
                    tensor.wait_ge(vs_sem, vs_pos[('a2', t)])
                    tensor.wait_ge(e_sems[t], 32)
                    if t > 1:
                        tensor.wait_ge(as_sem, as_pos[('a3', t - 2)])
                    tensor.matmul(pl1[t].ap()[0:32, :], tw0a, zA[:, C[t]],
                                  start=True, stop=False).then_inc(mm_sem)
                elif kind == 'l1b':
                    tensor.matmul(pl1[t].ap()[0:32, :], tw0b, zB[:, C[t]],
                                  start=False, stop=True).then_inc(mm_sem)
                elif kind == 'l2':
                    tensor.wait_ge(as_sem, as_pos[('a3', t)])
                    if t > 0:
                        tensor.wait_ge(vs_sem, vs_pos[('a4', t - 1)])
                    tensor.matmul(pl2.ap()[0:16, :], tw1, t1o[:, C[t]],
                                  start=True, stop=True).then_inc(mm_sem)
                else:  # l3
                    tensor.wait_ge(vs_sem, vs_pos[('a4', t)])
                    if t > 0:
                        tensor.wait_ge(as_sem, as_pos[('a5', t - 1)])
                    tensor.matmul(pl3.ap()[0:1, :], tw2, t2o[:, C[t]],
                                  start=True, stop=True).then_inc(mm_sem)

        @block.scalar
        def _(scalar):
            Relu = mybir.ActivationFunctionType.Relu
            Sigmoid = mybir.ActivationFunctionType.Sigmoid
            # weight+bias slab on the scalar HWDGE queue, parallel with xt +
            # embT chunks on sync
            scalar.dma_start(out=ws[:], in_=wslab[:]).then_inc(d1_sem, 16)
            scalar.wait_ge(z_sem, 1)
            scalar.activation(scr[0:1, 1:2], scr[0:1, 0:1], Sigmoid)
            for op in act_order:
                kind, t = op
                if kind == 'a1':
                    scalar.wait_ge(mm_sem, mm_pos[('b0', t)])
                    scalar.activation(h0[:, C[t]], pb0.ap()[0:8, :], Relu,
                                      bias=bb0).then_inc(as_sem)
                elif kind == 'a3':
                    scalar.wait_ge(mm_sem, mm_pos[('l1b', t)])
                    scalar.activation(t1o[:, C[t]], pl1[t].ap()[0:32, :], Relu,
                                      bias=tb0).then_inc(as_sem)
                else:  # a5
                    scalar.wait_ge(mm_sem, mm_pos[('l3', t)])
                    scalar.activation(o_s[:, C[t]], pl3.ap()[0:1, :], Sigmoid,
                                      bias=tb2).then_inc(as_sem)

        @block.vector
        def _(vector):
            add = mybir.AluOpType.add
            mx = mybir.AluOpType.max
            for op in dve_order:
                kind, t = op
                if kind == 'a2':
                    vector.wait_ge(mm_sem, mm_pos[('b1', t)])
                    vector.tensor_scalar(zA[0:8, C[t]], pb1.ap()[0:8, :],
                                         bb1, 0.0, add, mx).then_inc(vs_sem)
                else:  # a4
                    vector.wait_ge(mm_sem, mm_pos[('l2', t)])
                    vector.tensor_scalar(t2o[:, C[t]], pl2.ap()[0:16, :],
                                         tb1, 0.0, add, mx).then_inc(vs_sem)

    nc.compile()
    return nc


# ------------------------------------------------------------------ host ----

_CACHE = {}
MLP_F32 = False        # True: f32 MLP (safer accuracy, ~+20% HW time)
TRACE = False          # set True (with BASS_PERFETTO_PROFILE_ALL_CORES=1)
LAST_EXEC_NS = {}      # pass name -> exec_time_ns of slowest core


def _get_nc(key, builder, *args):
    if key not in _CACHE:
        _CACHE[key] = builder(*args)
    return _CACHE[key]


def kernel(dense_x, sparse_idx, sparse_offsets, hash_seeds, emb_table,
           bot_w0, bot_b0, bot_w1, bot_b1,
           top_w0, top_b0, top_w1, top_b1, top_w2, top_b2):
    dense_x = np.asarray(dense_x, np.float32)
    sparse_idx = np.asarray(sparse_idx, np.int64)
    offs = np.asarray(sparse_offsets, np.int64)
    if not np.array_equal(offs, np.arange(B, dtype=np.int64)):
        raise NotImplementedError("kernel assumes one-index bags "
                                  "(sparse_offsets == arange(B))")
    hash_seeds = np.asarray(hash_seeds, np.int64)
    emb_table = np.asarray(emb_table, np.float32)

    # ---- hash + route (host: routing metadata only)
    rows = np.empty((N_FEAT, B), np.int64)
    for i in range(N_FEAT):
        rows[i] = _hash_mod(sparse_idx[i], hash_seeds[i], V)
    flat = rows.reshape(-1)                       # j = i*B + b
    core = flat // ROWS_PER_CORE                  # owning core
    lr = flat - core * ROWS_PER_CORE              # row within shard
    win = lr // WIN
    li = (lr % WIN).astype(np.int16)              # window-local row

    # slot k of lookup j within its (core, win) bucket, ordered by row id
    # (sorted rows give the SDMA reads page locality)
    bucket = (core * N_WIN + win).astype(np.int64)
    order = np.argsort(bucket * WIN + li, kind="stable")
    sorted_b = bucket[order]
    seg_start = np.searchsorted(sorted_b, np.arange(N_CORES * N_WIN))
    slot_sorted = np.arange(flat.size) - seg_start[sorted_b]
    slot = np.empty(flat.size, np.int64)
    slot[order] = slot_sorted
    counts = np.bincount(bucket, minlength=N_CORES * N_WIN).reshape(
        N_CORES, N_WIN)
    nk = tuple(max(1, int((counts[:, w].max() + 127) // 128))
               for w in range(N_WIN))
    ioff = np.concatenate([[0], np.cumsum([k * 8 for k in nk])])
    goff = np.concatenate([[0], np.cumsum([k * D for k in nk])])

    # ---- pass 1 inputs; pad slots get spread indices (all-same-row padding
    # would hammer one HBM line and serialize that core's transfers).
    # (-1 trim padding was tried: it hangs the NEFF — completion sem never
    # fires for trimmed descriptor batches under this runtime.)
    idx_flat = np.empty((N_CORES, 16, int(ioff[-1])), np.int16)
    for w in range(N_WIN):
        k = nk[w] * 128
        spread = ((np.arange(k, dtype=np.int64) * 2287) % WIN).astype(np.int16)
        blk = spread.reshape(k // 16, 16).T           # [16, icols_w]
        idx_flat[:, :, int(ioff[w]):int(ioff[w + 1])] = blk[None]
    # place idx j of (c, w) at [j % 16, ioff[w] + j // 16]
    icol = (ioff[win] + slot // 16).astype(np.int64)
    idx_flat[core, slot % 16, icol] = li
    idx_in = np.ascontiguousarray(
        np.broadcast_to(idx_flat[:, None], (N_CORES, 8, 16, int(ioff[-1])))
        .reshape(N_CORES, 128, int(ioff[-1])))

    pad_rows = N_CORES * ROWS_PER_CORE - V
    table_pad = np.concatenate(
        [emb_table, np.zeros((pad_rows, D), np.float32)], axis=0)

    nc1 = _get_nc(("gather", nk), _build_gather_nc, nk)
    in_maps1 = [
        {"table": table_pad[c * ROWS_PER_CORE:(c + 1) * ROWS_PER_CORE],
         "idx": idx_in[c]}
        for c in range(N_CORES)
    ]
    r1 = run_bass_kernel_spmd(nc1, in_maps1, list(range(N_CORES)), trace=TRACE)
    LAST_EXEC_NS["gather"] = r1.exec_time_ns
    res1 = r1.results

    # ---- reassemble: emb[j] = gout[core][slot%128, goff[win] + (slot//128)*D :]
    bf16 = np.float32 if MLP_F32 else ml_dtypes.bfloat16
    emb = np.empty((N_FEAT * B, D), ml_dtypes.bfloat16)
    gcol = (goff[win] + (slot // 128) * D).astype(np.int64)
    for c in range(N_CORES):
        g = res1[c]["gout"]
        m = core == c
        emb[m] = g[(slot[m] % 128)[:, None], gcol[m][:, None] + np.arange(D)]
    emb = emb.reshape(N_FEAT, N_CORES, B_SH, D)

    # ---- pass 2 inputs (bf16/f32 activations+weights, f32 biases packed
    # into the slab tail as bitcast columns)
    xt_all = np.ascontiguousarray(
        dense_x.T.reshape(16, N_CORES, B_SH).transpose(1, 0, 2)).astype(bf16)
    embT_all = np.ascontiguousarray(
        emb.transpose(1, 0, 3, 2).reshape(N_CORES, N_FEAT * D, B_SH)
    ).astype(bf16)

    wcols = 103 if MLP_F32 else 108
    wslab = np.zeros((128, wcols), bf16)
    wslab[0:16, 0:8] = np.asarray(bot_w0, np.float32).astype(bf16)
    wslab[0:8, 8:16] = np.asarray(bot_w1, np.float32).astype(bf16)
    tw0f = np.asarray(top_w0, np.float32).astype(bf16)
    wslab[0:128, 16:48] = tw0f[0:128]
    wslab[0:72, 48:80] = tw0f[128:200]
    wslab[0:32, 80:96] = np.asarray(top_w1, np.float32).astype(bf16)
    wslab[0:16, 96:97] = np.asarray(top_w2, np.float32).astype(bf16)
    bslab = np.zeros((128, 5), np.float32)
    bslab[0:8, 0] = np.asarray(bot_b0, np.float32)
    bslab[0:8, 1] = np.asarray(bot_b1, np.float32)
    bslab[0:32, 2] = np.asarray(top_b0, np.float32)
    bslab[0:16, 3] = np.asarray(top_b1, np.float32)
    bslab[0:1, 4] = np.asarray(top_b2, np.float32)
    if MLP_F32:
        wslab[:, 98:103] = bslab
    else:
        wslab[:, 98:108] = bslab.view(np.uint16).view(ml_dtypes.bfloat16)
    weights = {"wslab": wslab}
    nc2 = _get_nc(("mlp", MLP_F32), _build_mlp_nc,
                  F32 if MLP_F32 else BF16)
    in_maps2 = [
        {"xt": xt_all[c], "embT": embT_all[c], **weights}
        for c in range(N_CORES)
    ]
    r2 = run_bass_kernel_spmd(nc2, in_maps2, list(range(N_CORES)), trace=TRACE)
    LAST_EXEC_NS["mlp"] = r2.exec_time_ns
    res2 = r2.results

    out = np.concatenate([res2[c]["out"][0] for c in range(N_CORES)])
    return out.reshape(B, 1).astype(np.float64)



# revision 16
# speedup vs baseline: 1.1053x; 1.1053x over previous
"""MicroDLRM (hash-embedding DLRM) on 8 TRN2 NeuronCores.

Strategy (per sharding hint): row-shard the 512MB embedding table across the
8 cores (262144 rows each); the host computes the splitmix64 hash (routing
metadata) and buckets each lookup to its owning (core, 32K-row window); each
core gathers its rows from its own HBM table shard with one dma_gather per
window (int16 window-local indices, -1-padded so the Q7 ucode trims the tail).
A second data-parallel pass computes the bot/top MLPs on a 2048-row batch
shard per core in bf16 (f32 accumulate).  The host only does hashing, index
routing, and the inter-pass permutation (the all-to-all of the hint), never
touching table payload beyond shipping contiguous shards.
"""

import sys

sys.path.insert(0, "/opt/trn_rl_repo")

from contextlib import ExitStack

import ml_dtypes
import numpy as np

import concourse.bass as bass
import concourse.bacc as bacc
import concourse.mybir as mybir
from concourse.bass_utils import run_bass_kernel_spmd
from concourse import library_config

N_CORES = 8
V = 2_000_000
D = 64                      # embedding width
B = 16384
N_FEAT = 3
B_SH = B // N_CORES         # 2048 batch rows per core
ROWS_PER_CORE = 262144      # 8 x 262144 = 2,097,152 >= V (last core padded)
WIN = 32768                 # int16-addressable window
N_WIN = ROWS_PER_CORE // WIN
N_GQ = 4             # gather SWDGE queues to round-robin (Q7 pairs)
BF16 = mybir.dt.bfloat16
F32 = mybir.dt.float32

_C1 = np.int64(13787848793156543929 - (1 << 64))
_C2 = np.int64(10723151780598845931 - (1 << 64))


def _hash_mod(idx: np.ndarray, seed: np.int64, mod: int) -> np.ndarray:
    with np.errstate(over="ignore"):
        x = idx.astype(np.int64) ^ seed
        x = (x ^ (x >> np.int64(30))) * _C1
        x = (x ^ (x >> np.int64(27))) * _C2
        x = x ^ (x >> np.int64(31))
    return np.abs(x) % np.int64(mod)


# ---------------------------------------------------------------- pass 1 ----


def _build_gather_nc(nk: tuple, caps: tuple):
    """Per core: for window w, gather caps[w] (<= nk[w]*128) rows of D floats
    from the window's 32K-row slice of the local table shard into SBUF,
    convert to bf16 on the scalar engine (halves the store traffic), then
    store to DRAM.  The bot MLP for this core's batch shard rides along on
    the otherwise-idle PE/DVE engines.
    idx layout per dma_gather contract: idx j of a window lives at
    [j % 16, j // 16] in 16 partitions, replicated 8x across the 128
    partitions (one copy per Q7 core pair)."""
    nc = bacc.Bacc("TRN2", target_bir_lowering=False, num_devices=N_CORES,
                   debug=False, num_swdge_queues=N_GQ)
    ioff = [0]          # idx column offsets per window (128-slot granular)
    goff = [0]          # gather-out free-dim offsets per window (in elems)
    for k in nk:
        ioff.append(ioff[-1] + k * 8)      # cap/16 columns
        goff.append(goff[-1] + k * D)      # cap/128 slots * D
    table = nc.declare_dram_parameter(
        "table", [ROWS_PER_CORE, D], F32, isOutput=False)
    idx = nc.declare_dram_parameter(
        "idx", [128, ioff[-1]], mybir.dt.int16, isOutput=False)
    xt = nc.declare_dram_parameter("xt", [16, B_SH], BF16, isOutput=False)
    wbot = nc.declare_dram_parameter("wbot", [16, 24], BF16, isOutput=False)
    gout = nc.declare_dram_parameter(
        "gout", [128, goff[-1]], BF16, isOutput=True)
    h1 = nc.declare_dram_parameter("h1", [8, B_SH], BF16, isOutput=True)

    NT = B_SH // 512

    with ExitStack() as ctx:
        e = ctx.enter_context
        idx_s = e(nc.sbuf_tensor([128, ioff[-1]], mybir.dt.int16))
        gt = e(nc.sbuf_tensor([128, goff[-1]], F32))
        gtb = e(nc.sbuf_tensor([128, goff[-1]], BF16))
        xt_s = e(nc.sbuf_tensor([16, B_SH], BF16))
        ws = e(nc.sbuf_tensor([16, 24], BF16))
        h0 = e(nc.sbuf_tensor([8, B_SH], BF16))
        h1s = e(nc.sbuf_tensor([8, B_SH], BF16))
        pb0 = e(nc.psum_tensor("pb0", [128, 512], F32))
        pb1 = e(nc.psum_tensor("pb1", [128, 512], F32))
        i_sems = [e(nc.semaphore(f"i_sem{w}")) for w in range(N_WIN)]
        g_sems = [e(nc.semaphore(f"g_sem{w}")) for w in range(N_WIN)]
        v_sems = [e(nc.semaphore(f"v_sem{w}")) for w in range(N_WIN)]
        s_sem = e(nc.semaphore("s_sem"))
        wx_sem = e(nc.semaphore("wx_sem"))
        mm_sem = e(nc.semaphore("mm_sem"))
        a1_sem = e(nc.semaphore("a1_sem"))
        a2_sem = e(nc.semaphore("a2_sem"))
        h_sem = e(nc.semaphore("h_sem"))
        block = e(nc.Block())
        tab_ap = table.ap()

        wb0 = ws[0:16, 0:8]
        wb1 = ws[0:8, 8:16]
        bb0 = ws[0:8, 16:18].bitcast(F32)
        bb1 = ws[0:8, 18:20].bitcast(F32)
        C = [slice(t * 512, (t + 1) * 512) for t in range(NT)]

        def gview(ap, w):
            return bass.AP(ap.tensor, ap.offset + goff[w],
                           [ap.ap[0], [D, nk[w]], [1, D]])

        @block.gpsimd
        def _(gpsimd):
            gpsimd.load_library(library_config.mlp)
            for w in range(N_WIN):
                gpsimd.wait_ge(i_sems[w], 16)
                gpsimd.dma_gather(
                    out_ap=gview(gt.ap(), w),
                    in_ap=tab_ap[w * WIN:(w + 1) * WIN, :],
                    idxs_ap=idx_s[:, ioff[w]:ioff[w + 1]],
                    num_idxs=caps[w],
                    num_idxs_reg=caps[w],
                    elem_size=D,
                    single_packet=False,
                    queue_num=w % N_GQ,
                ).then_inc(g_sems[w], 16)

        @block.tensor
        def _(tensor):
            # bot MLP rides along under the gather
            tensor.wait_ge(wx_sem, 32)
            for t in range(NT):
                tensor.matmul(pb0.ap()[0:8, :], wb0, xt_s[:, C[t]],
                              start=True, stop=True).then_inc(mm_sem)
                if t > 0:
                    tensor.wait_ge(a2_sem, t)
                tensor.wait_ge(a1_sem, t + 1)
                tensor.matmul(pb1.ap()[0:8, :], wb1, h0[:, C[t]],
                              start=True, stop=True).then_inc(mm_sem)

        @block.vector
        def _(vector):
            add = mybir.AluOpType.add
            mx = mybir.AluOpType.max
            for t in range(NT):
                vector.wait_ge(mm_sem, 2 * t + 1)
                vector.tensor_scalar(h0[:, C[t]], pb0.ap()[0:8, :],
                                     bb0, 0.0, add, mx).then_inc(a1_sem)
                vector.wait_ge(mm_sem, 2 * t + 2)
                vector.tensor_scalar(h1s[:, C[t]], pb1.ap()[0:8, :],
                                     bb1, 0.0, add, mx).then_inc(a2_sem)

        @block.scalar
        def _(scalar):
            for w in range(N_WIN):
                scalar.wait_ge(g_sems[w], 16)
                scalar.copy(gtb[:, goff[w]:goff[w + 1]],
                            gt[:, goff[w]:goff[w + 1]]).then_inc(v_sems[w], 1)

        @block.sync
        def _(sync):
            for w in range(N_WIN):
                sync.dma_start(out=idx_s[:, ioff[w]:ioff[w + 1]],
                               in_=idx.ap()[:, ioff[w]:ioff[w + 1]]
                               ).then_inc(i_sems[w], 16)
            sync.dma_start(out=xt_s[:], in_=xt[:]).then_inc(wx_sem, 16)
            sync.dma_start(out=ws[:], in_=wbot[:]).then_inc(wx_sem, 16)
            for w in range(N_WIN):
                sync.wait_ge(v_sems[w], 1)
                sync.dma_start(out=gview(gout.ap(), w),
                               in_=gview(gtb.ap(), w)).then_inc(s_sem, 16)
            sync.wait_ge(a2_sem, NT)
            sync.dma_start(out=h1.ap()[:], in_=h1s[:]).then_inc(h_sem, 16)
            sync.wait_ge(s_sem, 16 * N_WIN)
            sync.wait_ge(h_sem, 16)

    nc.compile()
    return nc


# ---------------------------------------------------------------- pass 2 ----


def _build_mlp_nc(adt=BF16):
    """Per core: top MLP on concat(h1, embT) for the core's batch shard.
    The bot MLP ran in pass 1 (h1 input).  Activations kept as
    [feature, batch] in bf16; weights stationary on PE; f32 PSUM accumulate;
    bias+relu on Scalar/Vector engines, software-pipelined across the four
    512-column tiles."""
    nc = bacc.Bacc("TRN2", target_bir_lowering=False, num_devices=N_CORES,
                   debug=False)
    # biases ride in the weight slab (bitcast to f32 cols 98+); the slab and
    # h1 go on the sync queue ahead of the embT chunks (the scalar queue is
    # blocked by ACT table loads at block head)
    wcols = 108 if adt == BF16 else 103
    h1 = nc.declare_dram_parameter("h1", [8, B_SH], adt, isOutput=False)
    embT = nc.declare_dram_parameter(
        "embT", [N_FEAT * D, B_SH], adt, isOutput=False)
    wslab = nc.declare_dram_parameter(
        "wslab", [128, wcols], adt, isOutput=False)
    out = nc.declare_dram_parameter("out", [1, B_SH], F32, isOutput=True)

    NT = B_SH // 512  # 4 column tiles

    with ExitStack() as ctx:
        e = ctx.enter_context
        zA = e(nc.sbuf_tensor([128, B_SH], adt))  # 0:8 h1, 8:128 emb
        zB = e(nc.sbuf_tensor([72, B_SH], adt))   # emb rows 120:192
        t1o = e(nc.sbuf_tensor([32, B_SH], adt))
        t2o = e(nc.sbuf_tensor([16, B_SH], adt))
        o_s = e(nc.sbuf_tensor([1, B_SH], F32))
        ws = e(nc.sbuf_tensor([128, wcols], adt))
        bs = ws  # biases live in the tail columns of the weight slab
        pl1a = e(nc.psum_tensor("pl1a", [128, 512], F32))
        pl1b = e(nc.psum_tensor("pl1b", [128, 512], F32))
        pl2a = e(nc.psum_tensor("pl2a", [128, 512], F32))
        pl2b = e(nc.psum_tensor("pl2b", [128, 512], F32))
        pl3a = e(nc.psum_tensor("pl3a", [128, 512], F32))
        pl3b = e(nc.psum_tensor("pl3b", [128, 512], F32))
        d1_sem = e(nc.semaphore("d1_sem"))
        e_sems = [e(nc.semaphore(f"e_sem{t}")) for t in range(NT)]
        mm_sem = e(nc.semaphore("mm_sem"))
        as_sem = e(nc.semaphore("as_sem"))   # scalar acts: a3 x4, a5 x4
        vs_sem = e(nc.semaphore("vs_sem"))   # vector acts: a4 x4
        o_sem = e(nc.semaphore("o_sem"))
        block = e(nc.Block())

        tw0a = ws[0:128, 16:48]
        tw0b = ws[0:72, 48:80]
        tw1 = ws[0:32, 80:96]
        tw2 = ws[0:16, 96:97]
        if adt == BF16:
            def bias(p, k):  # f32 bias k bitcast from bf16 cols 98+2k
                return bs[0:p, 98 + 2 * k:100 + 2 * k].bitcast(F32)
        else:
            def bias(p, k):
                return bs[0:p, 98 + k:99 + k]
        tb0 = bias(32, 2)
        tb1 = bias(16, 3)
        tb2 = bias(1, 4)

        C = [slice(t * 512, (t + 1) * 512) for t in range(NT)]
        pl1 = [pl1a, pl1b, pl1a, pl1b]
        pl2 = [pl2a, pl2b, pl2a, pl2b]
        pl3 = [pl3a, pl3b, pl3a, pl3b]
        # PE emission order and 1-based position of each matmul
        pe_order = [('l1a', 0), ('l1b', 0), ('l1a', 1), ('l1b', 1),
                    ('l2', 0), ('l1a', 2), ('l1b', 2), ('l2', 1),
                    ('l3', 0), ('l1a', 3), ('l1b', 3), ('l2', 2),
                    ('l3', 1), ('l2', 3), ('l3', 2), ('l3', 3)]
        mm_pos = {op: i + 1 for i, op in enumerate(pe_order)}
        act_order = [('a3', 0), ('a3', 1), ('a5', 0), ('a3', 2),
                     ('a3', 3), ('a5', 1), ('a5', 2), ('a5', 3)]
        as_pos = {op: i + 1 for i, op in enumerate(act_order)}
        vs_pos = {('a4', t): t + 1 for t in range(NT)}

        @block.sync
        def _(sync):
            sync.dma_start(out=ws[:], in_=wslab[:]).then_inc(d1_sem, 16)
            sync.dma_start(out=zA[0:8, :], in_=h1[:]).then_inc(d1_sem, 16)
            for t in range(NT):
                sync.dma_start(out=zA[8:128, C[t]],
                               in_=embT[0:120, C[t]]).then_inc(e_sems[t], 16)
                sync.dma_start(out=zB[:, C[t]],
                               in_=embT[120:192, C[t]]).then_inc(e_sems[t], 16)
            for t in range(NT):
                sync.wait_ge(as_sem, as_pos[('a5', t)])
                sync.dma_start(out=out.ap()[:, C[t]],
                               in_=o_s[:, C[t]]).then_inc(o_sem, 16)
            sync.wait_ge(o_sem, 16 * NT)

        @block.tensor
        def _(tensor):
            tensor.wait_ge(d1_sem, 2 * 16)
            for op in pe_order:
                kind, t = op
                if kind == 'l1a':
                    tensor.wait_ge(e_sems[t], 32)
                    if t > 1:
                        tensor.wait_ge(as_sem, as_pos[('a3', t - 2)])
                    tensor.matmul(pl1[t].ap()[0:32, :], tw0a, zA[:, C[t]],
                                  start=True, stop=False).then_inc(mm_sem)
                elif kind == 'l1b':
                    tensor.matmul(pl1[t].ap()[0:32, :], tw0b, zB[:, C[t]],
                                  start=False, stop=True).then_inc(mm_sem)
                elif kind == 'l2':
                    tensor.wait_ge(as_sem, as_pos[('a3', t)])
                    if t > 1:
                        tensor.wait_ge(vs_sem, vs_pos[('a4', t - 2)])
                    tensor.matmul(pl2[t].ap()[0:16, :], tw1, t1o[:, C[t]],
                                  start=True, stop=True).then_inc(mm_sem)
                else:  # l3
                    tensor.wait_ge(vs_sem, vs_pos[('a4', t)])
                    if t > 1:
                        tensor.wait_ge(as_sem, as_pos[('a5', t - 2)])
                    tensor.matmul(pl3[t].ap()[0:1, :], tw2, t2o[:, C[t]],
                                  start=True, stop=True).then_inc(mm_sem)

        @block.scalar
        def _(scalar):
            Relu = mybir.ActivationFunctionType.Relu
            Sigmoid = mybir.ActivationFunctionType.Sigmoid
            for op in act_order:
                kind, t = op
                if kind == 'a3':
                    scalar.wait_ge(mm_sem, mm_pos[('l1b', t)])
                    scalar.activation(t1o[:, C[t]], pl1[t].ap()[0:32, :], Relu,
                                      bias=tb0).then_inc(as_sem)
                else:  # a5
                    scalar.wait_ge(mm_sem, mm_pos[('l3', t)])
                    scalar.activation(o_s[:, C[t]], pl3[t].ap()[0:1, :],
                                      Sigmoid, bias=tb2).then_inc(as_sem)

        @block.vector
        def _(vector):
            add = mybir.AluOpType.add
            mx = mybir.AluOpType.max
            for t in range(NT):
                vector.wait_ge(mm_sem, mm_pos[('l2', t)])
                vector.tensor_scalar(t2o[:, C[t]], pl2[t].ap()[0:16, :],
                                     tb1, 0.0, add, mx).then_inc(vs_sem)

    nc.compile()
    return nc


# ------------------------------------------------------------------ host ----

_CACHE = {}
MLP_F32 = False        # True: f32 MLP (safer accuracy, ~+20% HW time)
TRACE = False          # set True (with BASS_PERFETTO_PROFILE_ALL_CORES=1)
LAST_EXEC_NS = {}      # pass name -> exec_time_ns of slowest core


def _get_nc(key, builder, *args):
    if key not in _CACHE:
        _CACHE[key] = builder(*args)
    return _CACHE[key]


def kernel(dense_x, sparse_idx, sparse_offsets, hash_seeds, emb_table,
           bot_w0, bot_b0, bot_w1, bot_b1,
           top_w0, top_b0, top_w1, top_b1, top_w2, top_b2):
    dense_x = np.asarray(dense_x, np.float32)
    sparse_idx = np.asarray(sparse_idx, np.int64)
    offs = np.asarray(sparse_offsets, np.int64)
    if not np.array_equal(offs, np.arange(B, dtype=np.int64)):
        raise NotImplementedError("kernel assumes one-index bags "
                                  "(sparse_offsets == arange(B))")
    hash_seeds = np.asarray(hash_seeds, np.int64)
    emb_table = np.asarray(emb_table, np.float32)

    # ---- hash + route (host: routing metadata only)
    rows = np.empty((N_FEAT, B), np.int64)
    for i in range(N_FEAT):
        rows[i] = _hash_mod(sparse_idx[i], hash_seeds[i], V)
    flat = rows.reshape(-1)                       # j = i*B + b
    core = flat // ROWS_PER_CORE                  # owning core
    lr = flat - core * ROWS_PER_CORE              # row within shard
    win = lr // WIN
    li = (lr % WIN).astype(np.int16)              # window-local row

    # slot k of lookup j within its (core, win) bucket, ordered by row id
    # (sorted rows give the SDMA reads page locality)
    bucket = (core * N_WIN + win).astype(np.int64)
    order = np.argsort(bucket * WIN + li, kind="stable")
    sorted_b = bucket[order]
    seg_start = np.searchsorted(sorted_b, np.arange(N_CORES * N_WIN))
    slot_sorted = np.arange(flat.size) - seg_start[sorted_b]
    slot = np.empty(flat.size, np.int64)
    slot[order] = slot_sorted
    counts = np.bincount(bucket, minlength=N_CORES * N_WIN).reshape(
        N_CORES, N_WIN)
    # descriptor count per window: max over cores rounded up to 32 (the
    # SBUF buffers stay 128-slot granular; only caps[w] rows are gathered)
    caps = tuple(max(128, int((counts[:, w].max() + 31) // 32 * 32))
                 for w in range(N_WIN))
    nk = tuple((c + 127) // 128 for c in caps)
    ioff = np.concatenate([[0], np.cumsum([k * 8 for k in nk])])
    goff = np.concatenate([[0], np.cumsum([k * D for k in nk])])

    # ---- pass 1 inputs; pad slots get spread indices (all-same-row padding
    # would hammer one HBM line and serialize that core's transfers).
    # (-1 trim padding was tried: it hangs the NEFF — completion sem never
    # fires for trimmed descriptor batches under this runtime.)
    idx_flat = np.empty((N_CORES, 16, int(ioff[-1])), np.int16)
    for w in range(N_WIN):
        k = nk[w] * 128
        spread = ((np.arange(k, dtype=np.int64) * 2287) % WIN).astype(np.int16)
        blk = spread.reshape(k // 16, 16).T           # [16, icols_w]
        idx_flat[:, :, int(ioff[w]):int(ioff[w + 1])] = blk[None]
    # place idx j of (c, w) at [j % 16, ioff[w] + j // 16]
    icol = (ioff[win] + slot // 16).astype(np.int64)
    idx_flat[core, slot % 16, icol] = li
    idx_in = np.ascontiguousarray(
        np.broadcast_to(idx_flat[:, None], (N_CORES, 8, 16, int(ioff[-1])))
        .reshape(N_CORES, 128, int(ioff[-1])))

    pad_rows = N_CORES * ROWS_PER_CORE - V
    table_pad = np.concatenate(
        [emb_table, np.zeros((pad_rows, D), np.float32)], axis=0)

    # bot-MLP inputs ride along in pass 1 (computed on the idle PE/DVE)
    xt_all = np.ascontiguousarray(
        dense_x.T.reshape(16, N_CORES, B_SH).transpose(1, 0, 2)
    ).astype(ml_dtypes.bfloat16)
    wbot = np.zeros((16, 24), ml_dtypes.bfloat16)
    wbot[0:16, 0:8] = np.asarray(bot_w0, np.float32).astype(ml_dtypes.bfloat16)
    wbot[0:8, 8:16] = np.asarray(bot_w1, np.float32).astype(ml_dtypes.bfloat16)
    bb = np.zeros((16, 2), np.float32)
    bb[0:8, 0] = np.asarray(bot_b0, np.float32)
    bb[0:8, 1] = np.asarray(bot_b1, np.float32)
    wbot[:, 16:20] = bb.view(np.uint16).view(ml_dtypes.bfloat16)

    nc1 = _get_nc(("gather", nk, caps), _build_gather_nc, nk, caps)
    in_maps1 = [
        {"table": table_pad[c * ROWS_PER_CORE:(c + 1) * ROWS_PER_CORE],
         "idx": idx_in[c], "xt": xt_all[c], "wbot": wbot}
        for c in range(N_CORES)
    ]
    r1 = run_bass_kernel_spmd(nc1, in_maps1, list(range(N_CORES)), trace=TRACE)
    LAST_EXEC_NS["gather"] = r1.exec_time_ns
    res1 = r1.results

    # ---- reassemble: emb[j] = gout[core][slot%128, goff[win] + (slot//128)*D :]
    bf16 = np.float32 if MLP_F32 else ml_dtypes.bfloat16
    emb = np.empty((N_FEAT * B, D), ml_dtypes.bfloat16)
    gcol = (goff[win] + (slot // 128) * D).astype(np.int64)
    for c in range(N_CORES):
        g = res1[c]["gout"]
        m = core == c
        emb[m] = g[(slot[m] % 128)[:, None], gcol[m][:, None] + np.arange(D)]
    emb = emb.reshape(N_FEAT, N_CORES, B_SH, D)

    # ---- pass 2 inputs (bf16/f32 activations+weights, f32 biases packed
    # into the slab tail as bitcast columns)
    embT_all = np.ascontiguousarray(
        emb.transpose(1, 0, 3, 2).reshape(N_CORES, N_FEAT * D, B_SH)
    ).astype(bf16)
    h1_all = [np.asarray(res1[c]["h1"]).astype(bf16) for c in range(N_CORES)]

    wcols = 103 if MLP_F32 else 108
    wslab = np.zeros((128, wcols), bf16)
    tw0f = np.asarray(top_w0, np.float32).astype(bf16)
    wslab[0:128, 16:48] = tw0f[0:128]
    wslab[0:72, 48:80] = tw0f[128:200]
    wslab[0:32, 80:96] = np.asarray(top_w1, np.float32).astype(bf16)
    wslab[0:16, 96:97] = np.asarray(top_w2, np.float32).astype(bf16)
    bslab = np.zeros((128, 5), np.float32)
    bslab[0:32, 2] = np.asarray(top_b0, np.float32)
    bslab[0:16, 3] = np.asarray(top_b1, np.float32)
    bslab[0:1, 4] = np.asarray(top_b2, np.float32)
    if MLP_F32:
        wslab[:, 98:103] = bslab
    else:
        wslab[:, 98:108] = bslab.view(np.uint16).view(ml_dtypes.bfloat16)
    weights = {"wslab": wslab}
    nc2 = _get_nc(("mlp", MLP_F32), _build_mlp_nc,
                  F32 if MLP_F32 else BF16)
    in_maps2 = [
        {"h1": h1_all[c], "embT": embT_all[c], **weights}
        for c in range(N_CORES)
    ]
    r2 = run_bass_kernel_spmd(nc2, in_maps2, list(range(N_CORES)), trace=TRACE)
    LAST_EXEC_NS["mlp"] = r2.exec_time_ns
    res2 = r2.results

    out = np.concatenate([res2[c]["out"][0] for c in range(N_CORES)])
    return out.reshape(B, 1).astype(np.float64)



# revision 21
# speedup vs baseline: 1.1375x; 1.0291x over previous
"""MicroDLRM (hash-embedding DLRM) on 8 TRN2 NeuronCores.

Strategy (per sharding hint): row-shard the 512MB embedding table across the
8 cores (262144 rows each); the host computes the splitmix64 hash (routing
metadata) and buckets each lookup to its owning (core, 32K-row window); each
core gathers its rows from its own HBM table shard with one dma_gather per
window (int16 window-local indices, -1-padded so the Q7 ucode trims the tail).
A second data-parallel pass computes the bot/top MLPs on a 2048-row batch
shard per core in bf16 (f32 accumulate).  The host only does hashing, index
routing, and the inter-pass permutation (the all-to-all of the hint), never
touching table payload beyond shipping contiguous shards.
"""

import sys

sys.path.insert(0, "/opt/trn_rl_repo")

from contextlib import ExitStack

import ml_dtypes
import numpy as np

import concourse.bass as bass
import concourse.bacc as bacc
import concourse.mybir as mybir
from concourse.bass_utils import run_bass_kernel_spmd
from concourse import library_config

N_CORES = 8
V = 2_000_000
D = 64                      # embedding width
B = 16384
N_FEAT = 3
B_SH = B // N_CORES         # 2048 batch rows per core
ROWS_PER_CORE = 262144      # 8 x 262144 = 2,097,152 >= V (last core padded)
WIN = 32768                 # int16-addressable window
N_WIN = ROWS_PER_CORE // WIN
N_GQ = 4             # gather SWDGE queues to round-robin (Q7 pairs)
BF16 = mybir.dt.bfloat16
F32 = mybir.dt.float32

_C1 = np.int64(13787848793156543929 - (1 << 64))
_C2 = np.int64(10723151780598845931 - (1 << 64))


def _hash_mod(idx: np.ndarray, seed: np.int64, mod: int) -> np.ndarray:
    with np.errstate(over="ignore"):
        x = idx.astype(np.int64) ^ seed
        x = (x ^ (x >> np.int64(30))) * _C1
        x = (x ^ (x >> np.int64(27))) * _C2
        x = x ^ (x >> np.int64(31))
    return np.abs(x) % np.int64(mod)


# ---------------------------------------------------------------- pass 1 ----


def _build_gather_nc(nk: tuple, caps: tuple):
    """Per core: for window w, gather caps[w] (<= nk[w]*128) rows of D floats
    from the window's 32K-row slice of the local table shard into SBUF,
    convert to bf16 on the scalar engine (halves the store traffic), then
    store to DRAM.  The bot MLP for this core's batch shard rides along on
    the otherwise-idle PE/DVE engines.
    idx layout per dma_gather contract: idx j of a window lives at
    [j % 16, j // 16] in 16 partitions, replicated 8x across the 128
    partitions (one copy per Q7 core pair)."""
    nc = bacc.Bacc("TRN2", target_bir_lowering=False, num_devices=N_CORES,
                   debug=False, num_swdge_queues=N_GQ)
    ioff = [0]          # idx column offsets per window (128-slot granular)
    goff = [0]          # gather-out free-dim offsets per window (in elems)
    for k in nk:
        ioff.append(ioff[-1] + k * 8)      # cap/16 columns
        goff.append(goff[-1] + k * D)      # cap/128 slots * D
    table = nc.declare_dram_parameter(
        "table", [ROWS_PER_CORE, D], F32, isOutput=False)
    idx = nc.declare_dram_parameter(
        "idx", [128, ioff[-1]], mybir.dt.int16, isOutput=False)
    xt = nc.declare_dram_parameter("xt", [16, B_SH], BF16, isOutput=False)
    wbot = nc.declare_dram_parameter("wbot", [16, 24], BF16, isOutput=False)
    gout = nc.declare_dram_parameter(
        "gout", [128, goff[-1]], BF16, isOutput=True)
    h1 = nc.declare_dram_parameter("h1", [8, B_SH], BF16, isOutput=True)

    NT = B_SH // 512

    with ExitStack() as ctx:
        e = ctx.enter_context
        idx_s = e(nc.sbuf_tensor([128, ioff[-1]], mybir.dt.int16))
        gt = e(nc.sbuf_tensor([128, goff[-1]], F32))
        gtb = e(nc.sbuf_tensor([128, goff[-1]], BF16))
        xt_s = e(nc.sbuf_tensor([16, B_SH], BF16))
        ws = e(nc.sbuf_tensor([16, 24], BF16))
        h0 = e(nc.sbuf_tensor([8, B_SH], BF16))
        h1s = e(nc.sbuf_tensor([8, B_SH], BF16))
        pb0 = e(nc.psum_tensor("pb0", [128, 512], F32))
        pb1 = e(nc.psum_tensor("pb1", [128, 512], F32))
        i_sems = [e(nc.semaphore(f"i_sem{w}")) for w in range(N_WIN)]
        g_sems = [e(nc.semaphore(f"g_sem{w}")) for w in range(N_WIN)]
        v_sems = [e(nc.semaphore(f"v_sem{w}")) for w in range(N_WIN)]
        s_sem = e(nc.semaphore("s_sem"))
        wx_sem = e(nc.semaphore("wx_sem"))
        mm_sem = e(nc.semaphore("mm_sem"))
        a1_sem = e(nc.semaphore("a1_sem"))
        a2_sem = e(nc.semaphore("a2_sem"))
        h_sem = e(nc.semaphore("h_sem"))
        block = e(nc.Block())
        tab_ap = table.ap()

        wb0 = ws[0:16, 0:8]
        wb1 = ws[0:8, 8:16]
        bb0 = ws[0:8, 16:18].bitcast(F32)
        bb1 = ws[0:8, 18:20].bitcast(F32)
        C = [slice(t * 512, (t + 1) * 512) for t in range(NT)]

        def gview(ap, w):
            return bass.AP(ap.tensor, ap.offset + goff[w],
                           [ap.ap[0], [D, nk[w]], [1, D]])

        @block.gpsimd
        def _(gpsimd):
            gpsimd.load_library(library_config.mlp)
            for w in range(N_WIN):
                gpsimd.wait_ge(i_sems[w], 16)
                gpsimd.dma_gather(
                    out_ap=gview(gt.ap(), w),
                    in_ap=tab_ap[w * WIN:(w + 1) * WIN, :],
                    idxs_ap=idx_s[:, ioff[w]:ioff[w + 1]],
                    num_idxs=caps[w],
                    num_idxs_reg=caps[w],
                    elem_size=D,
                    single_packet=False,
                    queue_num=w % N_GQ,
                ).then_inc(g_sems[w], 16)

        @block.tensor
        def _(tensor):
            # bot MLP rides along under the gather
            tensor.wait_ge(wx_sem, 32)
            for t in range(NT):
                tensor.matmul(pb0.ap()[0:8, :], wb0, xt_s[:, C[t]],
                              start=True, stop=True).then_inc(mm_sem)
                if t > 0:
                    tensor.wait_ge(a2_sem, t)
                tensor.wait_ge(a1_sem, t + 1)
                tensor.matmul(pb1.ap()[0:8, :], wb1, h0[:, C[t]],
                              start=True, stop=True).then_inc(mm_sem)

        @block.vector
        def _(vector):
            add = mybir.AluOpType.add
            mx = mybir.AluOpType.max
            for t in range(NT):
                vector.wait_ge(mm_sem, 2 * t + 1)
                vector.tensor_scalar(h0[:, C[t]], pb0.ap()[0:8, :],
                                     bb0, 0.0, add, mx).then_inc(a1_sem)
                vector.wait_ge(mm_sem, 2 * t + 2)
                vector.tensor_scalar(h1s[:, C[t]], pb1.ap()[0:8, :],
                                     bb1, 0.0, add, mx).then_inc(a2_sem)

        @block.scalar
        def _(scalar):
            for w in range(N_WIN):
                scalar.wait_ge(g_sems[w], 16)
                scalar.copy(gtb[:, goff[w]:goff[w + 1]],
                            gt[:, goff[w]:goff[w + 1]]).then_inc(v_sems[w], 1)

        @block.sync
        def _(sync):
            for w in range(N_WIN):
                sync.dma_start(out=idx_s[:, ioff[w]:ioff[w + 1]],
                               in_=idx.ap()[:, ioff[w]:ioff[w + 1]]
                               ).then_inc(i_sems[w], 16)
            sync.dma_start(out=xt_s[:], in_=xt[:]).then_inc(wx_sem, 16)
            sync.dma_start(out=ws[:], in_=wbot[:]).then_inc(wx_sem, 16)
            for w in range(N_WIN - 1):
                sync.wait_ge(v_sems[w], 1)
                sync.dma_start(out=gview(gout.ap(), w),
                               in_=gview(gtb.ap(), w)).then_inc(s_sem, 16)
            # h1 store goes before the last window's store (bot MLP finishes
            # long before gather w7 does)
            sync.wait_ge(a2_sem, NT)
            sync.dma_start(out=h1.ap()[:], in_=h1s[:]).then_inc(h_sem, 16)
            sync.wait_ge(v_sems[N_WIN - 1], 1)
            sync.dma_start(out=gview(gout.ap(), N_WIN - 1),
                           in_=gview(gtb.ap(), N_WIN - 1)).then_inc(s_sem, 16)
            sync.wait_ge(s_sem, 16 * N_WIN)
            sync.wait_ge(h_sem, 16)

    nc.compile()
    return nc


# ---------------------------------------------------------------- pass 2 ----


def _build_mlp_nc(adt=BF16):
    """Per core: top MLP on concat(h1, embT) for the core's batch shard.
    The bot MLP ran in pass 1 (h1 input).  Activations kept as
    [feature, batch] in bf16; weights stationary on PE; f32 PSUM accumulate;
    bias+relu on Scalar/Vector engines, software-pipelined across the four
    512-column tiles."""
    nc = bacc.Bacc("TRN2", target_bir_lowering=False, num_devices=N_CORES,
                   debug=False)
    # biases ride in the weight slab (bitcast to f32 cols 98+); the slab and
    # h1 go on the sync queue ahead of the embT chunks (the scalar queue is
    # blocked by ACT table loads at block head)
    wcols = 108 if adt == BF16 else 103
    h1 = nc.declare_dram_parameter("h1", [8, B_SH], adt, isOutput=False)
    embT = nc.declare_dram_parameter(
        "embT", [N_FEAT * D, B_SH], adt, isOutput=False)
    wslab = nc.declare_dram_parameter(
        "wslab", [128, wcols], adt, isOutput=False)
    out = nc.declare_dram_parameter("out", [1, B_SH], F32, isOutput=True)

    NT = B_SH // 512  # 4 column tiles

    with ExitStack() as ctx:
        e = ctx.enter_context
        zA = e(nc.sbuf_tensor([128, B_SH], adt))  # 0:8 h1, 8:128 emb
        zB = e(nc.sbuf_tensor([72, B_SH], adt))   # emb rows 120:192
        t1o = e(nc.sbuf_tensor([32, B_SH], adt))
        t2o = e(nc.sbuf_tensor([16, B_SH], adt))
        o_s = e(nc.sbuf_tensor([1, B_SH], F32))
        ws = e(nc.sbuf_tensor([128, wcols], adt))
        bs = ws  # biases live in the tail columns of the weight slab
        pl1a = e(nc.psum_tensor("pl1a", [128, 512], F32))
        pl1b = e(nc.psum_tensor("pl1b", [128, 512], F32))
        pl2a = e(nc.psum_tensor("pl2a", [128, 512], F32))
        pl2b = e(nc.psum_tensor("pl2b", [128, 512], F32))
        pl3a = e(nc.psum_tensor("pl3a", [128, 512], F32))
        pl3b = e(nc.psum_tensor("pl3b", [128, 512], F32))
        d1_sem = e(nc.semaphore("d1_sem"))
        ea_sems = [e(nc.semaphore(f"ea_sem{t}")) for t in range(NT)]
        eb_sems = [e(nc.semaphore(f"eb_sem{t}")) for t in range(NT)]
        mm_sem = e(nc.semaphore("mm_sem"))
        as_sem = e(nc.semaphore("as_sem"))   # scalar acts: a3 x4, a5 x4
        vs_sem = e(nc.semaphore("vs_sem"))   # vector acts: a4 x4
        o_sem = e(nc.semaphore("o_sem"))
        block = e(nc.Block())

        tw0a = ws[0:128, 16:48]
        tw0b = ws[0:72, 48:80]
        tw1 = ws[0:32, 80:96]
        tw2 = ws[0:16, 96:97]
        if adt == BF16:
            def bias(p, k):  # f32 bias k bitcast from bf16 cols 98+2k
                return bs[0:p, 98 + 2 * k:100 + 2 * k].bitcast(F32)
        else:
            def bias(p, k):
                return bs[0:p, 98 + k:99 + k]
        tb0 = bias(32, 2)
        tb1 = bias(16, 3)
        tb2 = bias(1, 4)

        C = [slice(t * 512, (t + 1) * 512) for t in range(NT)]
        pl1 = [pl1a, pl1b, pl1a, pl1b]
        pl2 = [pl2a, pl2b, pl2a, pl2b]
        pl3 = [pl3a, pl3b, pl3a, pl3b]
        # PE emission order and 1-based position of each matmul
        pe_order = [('l1a', 0), ('l1b', 0), ('l1a', 1), ('l1b', 1),
                    ('l2', 0), ('l1a', 2), ('l1b', 2), ('l2', 1),
                    ('l3', 0), ('l1a', 3), ('l1b', 3), ('l2', 2),
                    ('l3', 1), ('l2', 3), ('l3', 2), ('l3', 3)]
        mm_pos = {op: i + 1 for i, op in enumerate(pe_order)}
        act_order = [('a3', 0), ('a3', 1), ('a5', 0), ('a3', 2),
                     ('a3', 3), ('a5', 1), ('a5', 2), ('a5', 3)]
        as_pos = {op: i + 1 for i, op in enumerate(act_order)}
        vs_pos = {('a4', t): t + 1 for t in range(NT)}

        @block.gpsimd
        def _(gpsimd):
            # weight slab + h1 on the gpsimd HWDGE queue, parallel with the
            # embT chunks on sync (the scalar queue is blocked by ACT table
            # loads at block head)
            gpsimd.dma_start(out=ws[:], in_=wslab[:]).then_inc(d1_sem, 16)
            gpsimd.dma_start(out=zA[0:8, :], in_=h1[:]).then_inc(d1_sem, 16)

        @block.sync
        def _(sync):
            for t in range(NT):
                sync.dma_start(out=zA[8:128, C[t]],
                               in_=embT[0:120, C[t]]).then_inc(ea_sems[t], 16)
                sync.dma_start(out=zB[:, C[t]],
                               in_=embT[120:192, C[t]]).then_inc(eb_sems[t], 16)
            for t in range(NT):
                sync.wait_ge(as_sem, as_pos[('a5', t)])
                sync.dma_start(out=out.ap()[:, C[t]],
                               in_=o_s[:, C[t]]).then_inc(o_sem, 16)
            sync.wait_ge(o_sem, 16 * NT)

        @block.tensor
        def _(tensor):
            tensor.wait_ge(d1_sem, 2 * 16)
            for op in pe_order:
                kind, t = op
                if kind == 'l1a':
                    tensor.wait_ge(ea_sems[t], 16)
                    if t > 1:
                        tensor.wait_ge(as_sem, as_pos[('a3', t - 2)])
                    tensor.matmul(pl1[t].ap()[0:32, :], tw0a, zA[:, C[t]],
                                  start=True, stop=False).then_inc(mm_sem)
                elif kind == 'l1b':
                    tensor.wait_ge(eb_sems[t], 16)
                    tensor.matmul(pl1[t].ap()[0:32, :], tw0b, zB[:, C[t]],
                                  start=False, stop=True).then_inc(mm_sem)
                elif kind == 'l2':
                    tensor.wait_ge(as_sem, as_pos[('a3', t)])
                    if t > 1:
                        tensor.wait_ge(vs_sem, vs_pos[('a4', t - 2)])
                    tensor.matmul(pl2[t].ap()[0:16, :], tw1, t1o[:, C[t]],
                                  start=True, stop=True).then_inc(mm_sem)
                else:  # l3
                    tensor.wait_ge(vs_sem, vs_pos[('a4', t)])
                    if t > 1:
                        tensor.wait_ge(as_sem, as_pos[('a5', t - 2)])
                    tensor.matmul(pl3[t].ap()[0:1, :], tw2, t2o[:, C[t]],
                                  start=True, stop=True).then_inc(mm_sem)

        @block.scalar
        def _(scalar):
            Relu = mybir.ActivationFunctionType.Relu
            Sigmoid = mybir.ActivationFunctionType.Sigmoid
            for op in act_order:
                kind, t = op
                if kind == 'a3':
                    scalar.wait_ge(mm_sem, mm_pos[('l1b', t)])
                    scalar.activation(t1o[:, C[t]], pl1[t].ap()[0:32, :], Relu,
                                      bias=tb0).then_inc(as_sem)
                else:  # a5
                    scalar.wait_ge(mm_sem, mm_pos[('l3', t)])
                    scalar.activation(o_s[:, C[t]], pl3[t].ap()[0:1, :],
                                      Sigmoid, bias=tb2).then_inc(as_sem)

        @block.vector
        def _(vector):
            add = mybir.AluOpType.add
            mx = mybir.AluOpType.max
            for t in range(NT):
                vector.wait_ge(mm_sem, mm_pos[('l2', t)])
                vector.tensor_scalar(t2o[:, C[t]], pl2[t].ap()[0:16, :],
                                     tb1, 0.0, add, mx).then_inc(vs_sem)

    nc.compile()
    return nc


# ------------------------------------------------------------------ host ----

_CACHE = {}
MLP_F32 = False        # True: f32 MLP (safer accuracy, ~+20% HW time)
TRACE = False          # set True (with BASS_PERFETTO_PROFILE_ALL_CORES=1)
LAST_EXEC_NS = {}      # pass name -> exec_time_ns of slowest core


def _get_nc(key, builder, *args):
    if key not in _CACHE:
        _CACHE[key] = builder(*args)
    return _CACHE[key]


def kernel(dense_x, sparse_idx, sparse_offsets, hash_seeds, emb_table,
           bot_w0, bot_b0, bot_w1, bot_b1,
           top_w0, top_b0, top_w1, top_b1, top_w2, top_b2):
    dense_x = np.asarray(dense_x, np.float32)
    sparse_idx = np.asarray(sparse_idx, np.int64)
    offs = np.asarray(sparse_offsets, np.int64)
    if not np.array_equal(offs, np.arange(B, dtype=np.int64)):
        raise NotImplementedError("kernel assumes one-index bags "
                                  "(sparse_offsets == arange(B))")
    hash_seeds = np.asarray(hash_seeds, np.int64)
    emb_table = np.asarray(emb_table, np.float32)

    # ---- hash + route (host: routing metadata only)
    rows = np.empty((N_FEAT, B), np.int64)
    for i in range(N_FEAT):
        rows[i] = _hash_mod(sparse_idx[i], hash_seeds[i], V)
    flat = rows.reshape(-1)                       # j = i*B + b
    core = flat // ROWS_PER_CORE                  # owning core
    lr = flat - core * ROWS_PER_CORE              # row within shard
    win = lr // WIN
    li = (lr % WIN).astype(np.int16)              # window-local row

    # slot k of lookup j within its (core, win) bucket, ordered by row id
    # (sorted rows give the SDMA reads page locality)
    bucket = (core * N_WIN + win).astype(np.int64)
    order = np.argsort(bucket * WIN + li, kind="stable")
    sorted_b = bucket[order]
    seg_start = np.searchsorted(sorted_b, np.arange(N_CORES * N_WIN))
    slot_sorted = np.arange(flat.size) - seg_start[sorted_b]
    slot = np.empty(flat.size, np.int64)
    slot[order] = slot_sorted
    counts = np.bincount(bucket, minlength=N_CORES * N_WIN).reshape(
        N_CORES, N_WIN)
    # descriptor count per window: max over cores rounded up to 32 (the
    # SBUF buffers stay 128-slot granular; only caps[w] rows are gathered)
    caps = tuple(max(128, int((counts[:, w].max() + 31) // 32 * 32))
                 for w in range(N_WIN))
    nk = tuple((c + 127) // 128 for c in caps)
    ioff = np.concatenate([[0], np.cumsum([k * 8 for k in nk])])
    goff = np.concatenate([[0], np.cumsum([k * D for k in nk])])

    # ---- pass 1 inputs; pad slots get spread indices (all-same-row padding
    # would hammer one HBM line and serialize that core's transfers).
    # (-1 trim padding was tried: it hangs the NEFF — completion sem never
    # fires for trimmed descriptor batches under this runtime.)
    idx_flat = np.empty((N_CORES, 16, int(ioff[-1])), np.int16)
    for w in range(N_WIN):
        k = nk[w] * 128
        spread = ((np.arange(k, dtype=np.int64) * 2287) % WIN).astype(np.int16)
        blk = spread.reshape(k // 16, 16).T           # [16, icols_w]
        idx_flat[:, :, int(ioff[w]):int(ioff[w + 1])] = blk[None]
    # place idx j of (c, w) at [j % 16, ioff[w] + j // 16]
    icol = (ioff[win] + slot // 16).astype(np.int64)
    idx_flat[core, slot % 16, icol] = li
    idx_in = np.ascontiguousarray(
        np.broadcast_to(idx_flat[:, None], (N_CORES, 8, 16, int(ioff[-1])))
        .reshape(N_CORES, 128, int(ioff[-1])))

    pad_rows = N_CORES * ROWS_PER_CORE - V
    table_pad = np.concatenate(
        [emb_table, np.zeros((pad_rows, D), np.float32)], axis=0)

    # bot-MLP inputs ride along in pass 1 (computed on the idle PE/DVE)
    xt_all = np.ascontiguousarray(
        dense_x.T.reshape(16, N_CORES, B_SH).transpose(1, 0, 2)
    ).astype(ml_dtypes.bfloat16)
    wbot = np.zeros((16, 24), ml_dtypes.bfloat16)
    wbot[0:16, 0:8] = np.asarray(bot_w0, np.float32).astype(ml_dtypes.bfloat16)
    wbot[0:8, 8:16] = np.asarray(bot_w1, np.float32).astype(ml_dtypes.bfloat16)
    bb = np.zeros((16, 2), np.float32)
    bb[0:8, 0] = np.asarray(bot_b0, np.float32)
    bb[0:8, 1] = np.asarray(bot_b1, np.float32)
    wbot[:, 16:20] = bb.view(np.uint16).view(ml_dtypes.bfloat16)

    nc1 = _get_nc(("gather", nk, caps), _build_gather_nc, nk, caps)
    in_maps1 = [
        {"table": table_pad[c * ROWS_PER_CORE:(c + 1) * ROWS_PER_CORE],
         "idx": idx_in[c], "xt": xt_all[c], "wbot": wbot}
        for c in range(N_CORES)
    ]
    r1 = run_bass_kernel_spmd(nc1, in_maps1, list(range(N_CORES)), trace=TRACE)
    LAST_EXEC_NS["gather"] = r1.exec_time_ns
    res1 = r1.results

    # ---- reassemble: emb[j] = gout[core][slot%128, goff[win] + (slot//128)*D :]
    bf16 = np.float32 if MLP_F32 else ml_dtypes.bfloat16
    emb = np.empty((N_FEAT * B, D), ml_dtypes.bfloat16)
    gcol = (goff[win] + (slot // 128) * D).astype(np.int64)
    for c in range(N_CORES):
        g = res1[c]["gout"]
        m = core == c
        emb[m] = g[(slot[m] % 128)[:, None], gcol[m][:, None] + np.arange(D)]
    emb = emb.reshape(N_FEAT, N_CORES, B_SH, D)

    # ---- pass 2 inputs (bf16/f32 activations+weights, f32 biases packed
    # into the slab tail as bitcast columns)
    embT_all = np.ascontiguousarray(
        emb.transpose(1, 0, 3, 2).reshape(N_CORES, N_FEAT * D, B_SH)
    ).astype(bf16)
    h1_all = [np.asarray(res1[c]["h1"]).astype(bf16) for c in range(N_CORES)]

    wcols = 103 if MLP_F32 else 108
    wslab = np.zeros((128, wcols), bf16)
    tw0f = np.asarray(top_w0, np.float32).astype(bf16)
    wslab[0:128, 16:48] = tw0f[0:128]
    wslab[0:72, 48:80] = tw0f[128:200]
    wslab[0:32, 80:96] = np.asarray(top_w1, np.float32).astype(bf16)
    wslab[0:16, 96:97] = np.asarray(top_w2, np.float32).astype(bf16)
    bslab = np.zeros((128, 5), np.float32)
    bslab[0:32, 2] = np.asarray(top_b0, np.float32)
    bslab[0:16, 3] = np.asarray(top_b1, np.float32)
    bslab[0:1, 4] = np.asarray(top_b2, np.float32)
    if MLP_F32:
        wslab[:, 98:103] = bslab
    else:
        wslab[:, 98:108] = bslab.view(np.uint16).view(ml_dtypes.bfloat16)
    weights = {"wslab": wslab}
    nc2 = _get_nc(("mlp", MLP_F32), _build_mlp_nc,
                  F32 if MLP_F32 else BF16)
    in_maps2 = [
        {"h1": h1_all[c], "embT": embT_all[c], **weights}
        for c in range(N_CORES)
    ]
    r2 = run_bass_kernel_spmd(nc2, in_maps2, list(range(N_CORES)), trace=TRACE)
    LAST_EXEC_NS["mlp"] = r2.exec_time_ns
    res2 = r2.results

    out = np.concatenate([res2[c]["out"][0] for c in range(N_CORES)])
    return out.reshape(B, 1).astype(np.float64)

